# revision 19
# baseline (speedup 1.0000x reference)
"""FENet (7-layer stride-2 conv feature extractor) on 8 Trainium2 NeuronCores.

Strategy
--------
The whole network is linear except the |.| at each feature tap, so each of the
8 output features is  feat_f(b) = scale_f * sum_j |A_f @ x_b|_j  for a
host-precomputed composite banded matrix A_f (built in fp64 from the conv
weights, including all the inter-layer zero padding).  On device, per core:

  1. Host pre-casts x to bf16 (padded 900 -> 1024 cols); DMA xbar-transpose
     loads [position, sample] tiles straight from HBM (positions must sit on
     the partition/contraction axis for matmul).  Halves HBM traffic and
     keeps the TensorEngine free of transpose work.
  2. Banded matmul in bf16: the stacked A rows (1168) are packed into 10
     row-blocks of <=128; per block only the k-chunks (128-column strips of
     the input positions) where the block has support are multiplied.
  3. ScalarE Abs (PSUM fp32 -> bf16), then a per-block 0/1 selector matmul
     (rows -> feature) accumulates all 8 features in PSUM; the exact fp32
     1/L_f mean divisors are applied on the PSUM->SBUF copy.
  4. Tiny PE transpose of the [8, 512] feature tile back to [sample, 8] and
     a contiguous DMA out.

Data parallel over batch: 24576 samples -> 8 cores x 3072.
"""

import os
import sys

import numpy as np

for _p in ("/opt/trn_rl_repo", os.path.expanduser("~/.axon_site/_ro/trn_rl_repo")):
    if os.path.isdir(_p) and _p not in sys.path:
        sys.path.insert(0, _p)

import concourse.bass as bass
import concourse.bacc as bacc
import concourse.mybir as mybir
from concourse import tile
from concourse.bass_utils import run_bass_kernel_spmd

F32 = mybir.dt.float32
BF16 = mybir.dt.bfloat16
NP_BF16 = mybir.dt.np(BF16)

N_CORES = 8
B_FULL = 24576
L_IN = 900
L_PAD = 1024                       # host zero-pads so every chunk is 128 wide
B_LOC = B_FULL // N_CORES          # 3072
N_TILE = 512                       # samples per matmul moving tile
N_GRP = 128                        # samples per transpose group
TILES = B_LOC // N_TILE            # 6
GRPS = N_TILE // N_GRP             # 4
NCH = 8
CHW = [128] * 7 + [4]              # valid contraction rows per chunk

KER, STR, PAD_L, PAD_R = 40, 2, 38, 39
N_LAYERS = 7


# ----------------------------------------------------------------- host math
def _conv_map(M, w):
    """M: [L, 900] map from x to current positions; returns conv(pad(M)) map."""
    Mp = np.pad(M, ((PAD_L, PAD_R), (0, 0)))
    Lo = (Mp.shape[0] - KER) // STR + 1
    out = np.zeros((Lo, M.shape[1]), dtype=M.dtype)
    for k in range(KER):
        out += w[k] * Mp[k : k + STR * Lo : STR, :]
    return out


def _build_composite(feat_w, pass_w):
    """[(A_f [L_f, 900] fp64, scale_f)] for the 8 features."""
    P = np.eye(L_IN, dtype=np.float64)
    maps = []
    for i in range(N_LAYERS):
        F = _conv_map(P, feat_w[i, 0, 0].astype(np.float64))
        maps.append((F, 1.0 / F.shape[0]))
        P = _conv_map(P, pass_w[i, 0, 0].astype(np.float64))
    maps.append((P, 1.0 / 32.0))     # 2**round(log2(45)) == 32
    return maps


def _pack_blocks(maps):
    """Pack A rows into <=128-row blocks; narrow features 0/1 get their own
    block runs, the wide rest are concatenated."""
    rows = []
    for fid, (A, sc) in enumerate(maps):
        for r in range(A.shape[0]):
            rows.append((fid, sc, A[r]))
    n0 = maps[0][0].shape[0]
    n1 = maps[1][0].shape[0]
    groups = [rows[:n0], rows[n0 : n0 + n1], rows[n0 + n1 :]]
    blocks = []
    for g in groups:
        for s in range(0, len(g), 128):
            blk = g[s : s + 128]
            M = np.stack([v for _, _, v in blk])
            chs = [c for c in range(NCH)
                   if np.any(M[:, c * 128 : min((c + 1) * 128, L_IN)] != 0.0)]
            blocks.append(dict(M=M, chunks=chs,
                               feats=[(f, sc) for f, sc, _ in blk]))
    return blocks


def _build_operands(blocks):
    """Device constants: stacked bf16 lhsT tiles, 0/1 bf16 selector tiles,
    and the fp32 per-feature scale vector."""
    n_mm = sum(len(b["chunks"]) for b in blocks)
    n_blk = len(blocks)
    wt = np.zeros((n_mm, 128, 128), dtype=np.float32)
    sel = np.zeros((n_blk, 128, 8), dtype=np.float32)
    fscale = np.zeros((8, 1), dtype=np.float32)
    sched = []                        # per block: (mrows, [(mm_idx, chunk)...])
    i = 0
    for b, blk in enumerate(blocks):
        mrows = blk["M"].shape[0]
        ent = []
        for c in blk["chunks"]:
            kw = CHW[c]
            wt[i, :kw, :mrows] = blk["M"][:, c * 128 : c * 128 + kw].T
            ent.append((i, c))
            i += 1
        for r, (f, sc) in enumerate(blk["feats"]):
            sel[b, r, f] = 1.0
            fscale[f, 0] = sc
        sched.append((mrows, ent))
    # device SBUF layout: partition-major [k, i, m] / [r, b, f]; one DMA each
    wt = np.ascontiguousarray(wt.transpose(1, 0, 2)).astype(NP_BF16)
    sel = np.ascontiguousarray(sel.transpose(1, 0, 2)).astype(NP_BF16)
    return wt, sel, fscale, sched


# ------------------------------------------------------------ device program
def _build_program(sched, n_mm, n_blk):
    nc = bacc.Bacc()
    xs_d = nc.dram_tensor("xs", [B_LOC, L_PAD], BF16, kind="ExternalInput")
    wt_d = nc.dram_tensor("wt", [128, n_mm, 128], BF16, kind="ExternalInput")
    sel_d = nc.dram_tensor("sel", [128, n_blk, 8], BF16, kind="ExternalInput")
    fs_d = nc.dram_tensor("fscale", [8, 1], F32, kind="ExternalInput")
    id_d = nc.dram_tensor("ident", [8, 8], F32, kind="ExternalInput")
    out_d = nc.dram_tensor("out", [B_LOC, 8], F32, kind="ExternalOutput")

    with tile.TileContext(nc) as tc:
        with (
            tc.tile_pool(name="const", bufs=1) as constp,
            tc.tile_pool(name="xt", bufs=1) as xtp,
            tc.tile_pool(name="za", bufs=3) as zap,
            tc.tile_pool(name="oute", bufs=2) as outp,
            tc.tile_pool(name="pz", bufs=4, space=bass.MemorySpace.PSUM) as pzp,
            tc.tile_pool(name="pf", bufs=2, space=bass.MemorySpace.PSUM) as pfp,
            tc.tile_pool(name="po", bufs=2, space=bass.MemorySpace.PSUM) as pop,
        ):
            wt_sb = constp.tile([128, n_mm, 128], BF16)
            nc.gpsimd.dma_start(wt_sb[:], wt_d[:])
            sel_sb = constp.tile([128, n_blk, 8], BF16)
            nc.gpsimd.dma_start(sel_sb[:], sel_d[:])
            fs_sb = constp.tile([8, 1], F32)
            nc.gpsimd.dma_start(fs_sb[:], fs_d[:])
            id_sb = constp.tile([8, 8], F32)
            nc.gpsimd.dma_start(id_sb[:], id_d[:])

            # ---- one whole-core xbar transpose per chunk (fixed ~1.2us
            # instruction cost dominates, so batch maximally)
            xt_chunks = []
            for c in range(NCH):
                xt_c = xtp.tile([128, B_LOC], BF16, tag=f"xt{c}")
                nc.sync.dma_start(
                    xt_c[:], xs_d[:, c * 128 : (c + 1) * 128], transpose=True)
                xt_chunks.append(xt_c)

            for t in range(TILES):
                trow = t * N_TILE

                # ---- banded matmuls, abs, per-block feature reduce
                pf = pfp.tile([8, N_TILE], F32, tag="pf")
                for b, (mrows, ent) in enumerate(sched):
                    pz = pzp.tile([mrows, N_TILE], F32, tag="pz")
                    for j, (i, c) in enumerate(ent):
                        kw = CHW[c]
                        nc.tensor.matmul(
                            pz[:],
                            wt_sb[0:kw, i, 0:mrows],
                            xt_chunks[c][0:kw, trow : trow + N_TILE],
                            start=(j == 0), stop=(j == len(ent) - 1),
                            skip_group_check=True)
                    za = zap.tile([mrows, N_TILE], BF16, tag="za")
                    nc.scalar.activation(
                        za[:], pz[:], mybir.ActivationFunctionType.Abs)
                    nc.tensor.matmul(
                        pf[:],
                        sel_sb[0:mrows, b, :],
                        za[:],
                        start=(b == 0), stop=(b == n_blk - 1),
                        skip_group_check=True)

                # ---- apply exact fp32 mean divisors, [8,512] -> [512,8], out
                fc = outp.tile([8, N_TILE], F32, tag="fc")
                nc.scalar.activation(
                    fc[:], pf[:], mybir.ActivationFunctionType.Copy,
                    scale=fs_sb[:])
                for g in range(GRPS):
                    row0 = (t * GRPS + g) * N_GRP
                    po = pop.tile([128, 8], F32, tag="po")
                    nc.tensor.transpose(
                        po[:], fc[:, g * N_GRP : (g + 1) * N_GRP], id_sb[:])
                    ob = outp.tile([128, 8], F32, tag="ob")
                    nc.vector.tensor_copy(ob[:], po[:])
                    nc.gpsimd.dma_start(out_d[row0 : row0 + N_GRP, :], ob[:])
    nc.finalize()
    return nc


_CACHE = {}


def _get_program(feat_w, pass_w):
    maps = _build_composite(feat_w, pass_w)
    blocks = _pack_blocks(maps)
    wt, sel, fscale, sched = _build_operands(blocks)
    key = tuple((m, tuple(e)) for m, e in sched)
    if key not in _CACHE:
        _CACHE[key] = _build_program(sched, wt.shape[1], sel.shape[1])
    return _CACHE[key], wt, sel, fscale


def kernel(x, feat_w, pass_w):
    nc, wt, sel, fscale = _get_program(feat_w, pass_w)
    ident = np.eye(8, dtype=np.float32)
    xs = np.zeros((B_FULL, L_PAD), dtype=NP_BF16)
    xs[:, :L_IN] = np.asarray(x, dtype=np.float32).reshape(B_FULL, L_IN)
    in_maps = [
        {"xs": xs[i * B_LOC : (i + 1) * B_LOC],
         "wt": wt, "sel": sel, "fscale": fscale, "ident": ident}
        for i in range(N_CORES)
    ]
    res = run_bass_kernel_spmd(nc, in_maps, list(range(N_CORES)))
    out = np.concatenate([res.results[i]["out"] for i in range(N_CORES)], axis=0)
    return np.ascontiguousarray(out.astype(np.float32))


# revision 20
# speedup vs baseline: 1.0254x; 1.0254x over previous
"""FENet (7-layer stride-2 conv feature extractor) on 8 Trainium2 NeuronCores.

Strategy
--------
The whole network is linear except the |.| at each feature tap, so each of the
8 output features is  feat_f(b) = scale_f * sum_j |A_f @ x_b|_j  for a
host-precomputed composite banded matrix A_f (built in fp64 from the conv
weights, including all the inter-layer zero padding).  On device, per core:

  1. Host pre-casts x to bf16 (padded 900 -> 1024 cols); DMA xbar-transpose
     loads [position, sample] tiles straight from HBM (positions must sit on
     the partition/contraction axis for matmul).  Halves HBM traffic and
     keeps the TensorEngine free of transpose work.
  2. Banded matmul in bf16: the stacked A rows (1168) are packed into 10
     row-blocks of <=128; per block only the k-chunks (128-column strips of
     the input positions) where the block has support are multiplied.
  3. ScalarE Abs (PSUM fp32 -> bf16), then a per-block 0/1 selector matmul
     (rows -> feature) accumulates all 8 features in PSUM; the exact fp32
     1/L_f mean divisors are applied on the PSUM->SBUF copy.
  4. Tiny PE transpose of the [8, 512] feature tile back to [sample, 8] and
     a contiguous DMA out.

Data parallel over batch: 24576 samples -> 8 cores x 3072.
"""

import os
import sys

import numpy as np

for _p in ("/opt/trn_rl_repo", os.path.expanduser("~/.axon_site/_ro/trn_rl_repo")):
    if os.path.isdir(_p) and _p not in sys.path:
        sys.path.insert(0, _p)

import concourse.bass as bass
import concourse.bacc as bacc
import concourse.mybir as mybir
from concourse import tile
from concourse.bass_utils import run_bass_kernel_spmd

F32 = mybir.dt.float32
BF16 = mybir.dt.bfloat16
NP_BF16 = mybir.dt.np(BF16)

N_CORES = 8
B_FULL = 24576
L_IN = 900
L_PAD = 1024                       # host zero-pads so every chunk is 128 wide
B_LOC = B_FULL // N_CORES          # 3072
N_TILE = 512                       # samples per matmul moving tile
N_GRP = 128                        # samples per transpose group
TILES = B_LOC // N_TILE            # 6
GRPS = N_TILE // N_GRP             # 4
NCH = 8
CHW = [128] * 7 + [4]              # valid contraction rows per chunk

KER, STR, PAD_L, PAD_R = 40, 2, 38, 39
N_LAYERS = 7


# ----------------------------------------------------------------- host math
def _conv_map(M, w):
    """M: [L, 900] map from x to current positions; returns conv(pad(M)) map."""
    Mp = np.pad(M, ((PAD_L, PAD_R), (0, 0)))
    Lo = (Mp.shape[0] - KER) // STR + 1
    out = np.zeros((Lo, M.shape[1]), dtype=M.dtype)
    for k in range(KER):
        out += w[k] * Mp[k : k + STR * Lo : STR, :]
    return out


def _build_composite(feat_w, pass_w):
    """[(A_f [L_f, 900] fp64, scale_f)] for the 8 features."""
    P = np.eye(L_IN, dtype=np.float64)
    maps = []
    for i in range(N_LAYERS):
        F = _conv_map(P, feat_w[i, 0, 0].astype(np.float64))
        maps.append((F, 1.0 / F.shape[0]))
        P = _conv_map(P, pass_w[i, 0, 0].astype(np.float64))
    maps.append((P, 1.0 / 32.0))     # 2**round(log2(45)) == 32
    return maps


def _pack_blocks(maps):
    """Pack A rows into <=128-row blocks; narrow features 0/1 get their own
    block runs, the wide rest are concatenated."""
    rows = []
    for fid, (A, sc) in enumerate(maps):
        for r in range(A.shape[0]):
            rows.append((fid, sc, A[r]))
    n0 = maps[0][0].shape[0]
    n1 = maps[1][0].shape[0]
    groups = [rows[:n0], rows[n0 : n0 + n1], rows[n0 + n1 :]]
    blocks = []
    for g in groups:
        for s in range(0, len(g), 128):
            blk = g[s : s + 128]
            M = np.stack([v for _, _, v in blk])
            chs = [c for c in range(NCH)
                   if np.any(M[:, c * 128 : min((c + 1) * 128, L_IN)] != 0.0)]
            blocks.append(dict(M=M, chunks=chs,
                               feats=[(f, sc) for f, sc, _ in blk]))
    return blocks


def _build_operands(blocks):
    """Device constants: stacked bf16 lhsT tiles, 0/1 bf16 selector tiles,
    and the fp32 per-feature scale vector."""
    n_mm = sum(len(b["chunks"]) for b in blocks)
    n_blk = len(blocks)
    wt = np.zeros((n_mm, 128, 128), dtype=np.float32)
    sel = np.zeros((n_blk, 128, 8), dtype=np.float32)
    fscale = np.zeros((8, 1), dtype=np.float32)
    sched = []                        # per block: (mrows, [(mm_idx, chunk)...])
    i = 0
    for b, blk in enumerate(blocks):
        mrows = blk["M"].shape[0]
        ent = []
        for c in blk["chunks"]:
            kw = CHW[c]
            wt[i, :kw, :mrows] = blk["M"][:, c * 128 : c * 128 + kw].T
            ent.append((i, c))
            i += 1
        for r, (f, sc) in enumerate(blk["feats"]):
            sel[b, r, f] = 1.0
            fscale[f, 0] = sc
        sched.append((mrows, ent))
    # device SBUF layout: partition-major [k, i, m] / [r, b, f]; one DMA each
    wt = np.ascontiguousarray(wt.transpose(1, 0, 2)).astype(NP_BF16)
    sel = np.ascontiguousarray(sel.transpose(1, 0, 2)).astype(NP_BF16)
    return wt, sel, fscale, sched


# ------------------------------------------------------------ device program
def _build_program(sched, n_mm, n_blk):
    nc = bacc.Bacc()
    xs_d = nc.dram_tensor("xs", [B_LOC, L_PAD], BF16, kind="ExternalInput")
    wt_d = nc.dram_tensor("wt", [128, n_mm, 128], BF16, kind="ExternalInput")
    sel_d = nc.dram_tensor("sel", [128, n_blk, 8], BF16, kind="ExternalInput")
    fs_d = nc.dram_tensor("fscale", [8, 1], F32, kind="ExternalInput")
    id_d = nc.dram_tensor("ident", [8, 8], F32, kind="ExternalInput")
    out_d = nc.dram_tensor("out", [B_LOC, 8], F32, kind="ExternalOutput")

    with tile.TileContext(nc) as tc:
        with (
            tc.tile_pool(name="const", bufs=1) as constp,
            tc.tile_pool(name="xt", bufs=1) as xtp,
            tc.tile_pool(name="za", bufs=6) as zap,
            tc.tile_pool(name="oute", bufs=2) as outp,
            tc.tile_pool(name="pz", bufs=4, space=bass.MemorySpace.PSUM) as pzp,
            tc.tile_pool(name="pf", bufs=2, space=bass.MemorySpace.PSUM) as pfp,
            tc.tile_pool(name="po", bufs=2, space=bass.MemorySpace.PSUM) as pop,
        ):
            wt_sb = constp.tile([128, n_mm, 128], BF16)
            nc.gpsimd.dma_start(wt_sb[:], wt_d[:])
            sel_sb = constp.tile([128, n_blk, 8], BF16)
            nc.gpsimd.dma_start(sel_sb[:], sel_d[:])
            fs_sb = constp.tile([8, 1], F32)
            nc.gpsimd.dma_start(fs_sb[:], fs_d[:])
            id_sb = constp.tile([8, 8], F32)
            nc.gpsimd.dma_start(id_sb[:], id_d[:])

            # ---- xbar transposes: instruction cost is mostly fixed, so
            # batch big; but split tile 0 out so convs start early
            xt_chunks = []
            for c in range(NCH):
                xt_c = xtp.tile([128, B_LOC], BF16, tag=f"xt{c}")
                nc.sync.dma_start(
                    xt_c[:, 0:N_TILE],
                    xs_d[0:N_TILE, c * 128 : (c + 1) * 128], transpose=True)
                xt_chunks.append(xt_c)
            for c in range(NCH):
                nc.sync.dma_start(
                    xt_chunks[c][:, N_TILE:B_LOC],
                    xs_d[N_TILE:B_LOC, c * 128 : (c + 1) * 128],
                    transpose=True)

            for t in range(TILES):
                trow = t * N_TILE

                # ---- banded matmuls, abs, per-block feature reduce
                pf = pfp.tile([8, N_TILE], F32, tag="pf")
                for b, (mrows, ent) in enumerate(sched):
                    pz = pzp.tile([mrows, N_TILE], F32, tag="pz")
                    for j, (i, c) in enumerate(ent):
                        kw = CHW[c]
                        nc.tensor.matmul(
                            pz[:],
                            wt_sb[0:kw, i, 0:mrows],
                            xt_chunks[c][0:kw, trow : trow + N_TILE],
                            start=(j == 0), stop=(j == len(ent) - 1),
                            skip_group_check=True)
                    za = zap.tile([mrows, N_TILE], BF16, tag="za")
                    nc.scalar.activation(
                        za[:], pz[:], mybir.ActivationFunctionType.Abs)
                    nc.tensor.matmul(
                        pf[:],
                        sel_sb[0:mrows, b, :],
                        za[:],
                        start=(b == 0), stop=(b == n_blk - 1),
                        skip_group_check=True)

                # ---- apply exact fp32 mean divisors, [8,512] -> [512,8], out
                fc = outp.tile([8, N_TILE], F32, tag="fc")
                nc.scalar.activation(
                    fc[:], pf[:], mybir.ActivationFunctionType.Copy,
                    scale=fs_sb[:])
                for g in range(GRPS):
                    row0 = (t * GRPS + g) * N_GRP
                    po = pop.tile([128, 8], F32, tag="po")
                    nc.tensor.transpose(
                        po[:], fc[:, g * N_GRP : (g + 1) * N_GRP], id_sb[:])
                    ob = outp.tile([128, 8], F32, tag="ob")
                    nc.vector.tensor_copy(ob[:], po[:])
                    nc.gpsimd.dma_start(out_d[row0 : row0 + N_GRP, :], ob[:])
    nc.finalize()
    return nc


_CACHE = {}


def _get_program(feat_w, pass_w):
    maps = _build_composite(feat_w, pass_w)
    blocks = _pack_blocks(maps)
    wt, sel, fscale, sched = _build_operands(blocks)
    key = tuple((m, tuple(e)) for m, e in sched)
    if key not in _CACHE:
        _CACHE[key] = _build_program(sched, wt.shape[1], sel.shape[1])
    return _CACHE[key], wt, sel, fscale


def kernel(x, feat_w, pass_w):
    nc, wt, sel, fscale = _get_program(feat_w, pass_w)
    ident = np.eye(8, dtype=np.float32)
    xs = np.zeros((B_FULL, L_PAD), dtype=NP_BF16)
    xs[:, :L_IN] = np.asarray(x, dtype=np.float32).reshape(B_FULL, L_IN)
    in_maps = [
        {"xs": xs[i * B_LOC : (i + 1) * B_LOC],
         "wt": wt, "sel": sel, "fscale": fscale, "ident": ident}
        for i in range(N_CORES)
    ]
    res = run_bass_kernel_spmd(nc, in_maps, list(range(N_CORES)))
    out = np.concatenate([res.results[i]["out"] for i in range(N_CORES)], axis=0)
    return np.ascontiguousarray(out.astype(np.float32))


# revision 21
# speedup vs baseline: 1.1333x; 1.1053x over previous
"""FENet (7-layer stride-2 conv feature extractor) on 8 Trainium2 NeuronCores.

Strategy
--------
The whole network is linear except the |.| at each feature tap, so each of the
8 output features is  feat_f(b) = scale_f * sum_j |A_f @ x_b|_j  for a
host-precomputed composite banded matrix A_f (built in fp64 from the conv
weights, including all the inter-layer zero padding).  On device, per core:

  1. Host pre-casts x to bf16 (padded 900 -> 1024 cols); DMA xbar-transpose
     loads [position, sample] tiles straight from HBM (positions must sit on
     the partition/contraction axis for matmul).  Halves HBM traffic and
     keeps the TensorEngine free of transpose work.
  2. Banded matmul in bf16: the stacked A rows (1168) are packed into 10
     row-blocks of <=128; per block only the k-chunks (128-column strips of
     the input positions) where the block has support are multiplied.
  3. ScalarE Abs (PSUM fp32 -> bf16), then a per-block 0/1 selector matmul
     (rows -> feature) accumulates all 8 features in PSUM; the exact fp32
     1/L_f mean divisors are applied on the PSUM->SBUF copy.
  4. Tiny PE transpose of the [8, 512] feature tile back to [sample, 8] and
     a contiguous DMA out.

Data parallel over batch: 24576 samples -> 8 cores x 3072.
"""

import os
import sys

import numpy as np

for _p in ("/opt/trn_rl_repo", os.path.expanduser("~/.axon_site/_ro/trn_rl_repo")):
    if os.path.isdir(_p) and _p not in sys.path:
        sys.path.insert(0, _p)

import concourse.bass as bass
import concourse.bacc as bacc
import concourse.mybir as mybir
from concourse import tile
from concourse.bass_utils import run_bass_kernel_spmd

F32 = mybir.dt.float32
BF16 = mybir.dt.bfloat16
NP_BF16 = mybir.dt.np(BF16)

N_CORES = 8
B_FULL = 24576
L_IN = 900
L_PAD = 1024                       # host zero-pads so every chunk is 128 wide
B_LOC = B_FULL // N_CORES          # 3072
N_TILE = 512                       # samples per matmul moving tile
N_GRP = 128                        # samples per transpose group
TILES = B_LOC // N_TILE            # 6
GRPS = N_TILE // N_GRP             # 4
NCH = 8
CHW = [128] * 7 + [4]              # valid contraction rows per chunk

KER, STR, PAD_L, PAD_R = 40, 2, 38, 39
N_LAYERS = 7


# ----------------------------------------------------------------- host math
def _conv_map(M, w):
    """M: [L, 900] map from x to current positions; returns conv(pad(M)) map."""
    Mp = np.pad(M, ((PAD_L, PAD_R), (0, 0)))
    Lo = (Mp.shape[0] - KER) // STR + 1
    out = np.zeros((Lo, M.shape[1]), dtype=M.dtype)
    for k in range(KER):
        out += w[k] * Mp[k : k + STR * Lo : STR, :]
    return out


def _build_composite(feat_w, pass_w):
    """[(A_f [L_f, 900] fp64, scale_f)] for the 8 features."""
    P = np.eye(L_IN, dtype=np.float64)
    maps = []
    for i in range(N_LAYERS):
        F = _conv_map(P, feat_w[i, 0, 0].astype(np.float64))
        maps.append((F, 1.0 / F.shape[0]))
        P = _conv_map(P, pass_w[i, 0, 0].astype(np.float64))
    maps.append((P, 1.0 / 32.0))     # 2**round(log2(45)) == 32
    return maps


def _pack_blocks(maps):
    """Pack A rows into <=128-row blocks; narrow features 0/1 get their own
    block runs, the wide rest are concatenated."""
    rows = []
    for fid, (A, sc) in enumerate(maps):
        for r in range(A.shape[0]):
            rows.append((fid, sc, A[r]))
    n0 = maps[0][0].shape[0]
    n1 = maps[1][0].shape[0]
    groups = [rows[:n0], rows[n0 : n0 + n1], rows[n0 + n1 :]]
    blocks = []
    for g in groups:
        for s in range(0, len(g), 128):
            blk = g[s : s + 128]
            M = np.stack([v for _, _, v in blk])
            chs = [c for c in range(NCH)
                   if np.any(M[:, c * 128 : min((c + 1) * 128, L_IN)] != 0.0)]
            blocks.append(dict(M=M, chunks=chs,
                               feats=[(f, sc) for f, sc, _ in blk]))
    return blocks


def _build_operands(blocks):
    """Device constants: stacked bf16 lhsT tiles, 0/1 bf16 selector tiles,
    and the fp32 per-feature scale vector."""
    n_mm = sum(len(b["chunks"]) for b in blocks)
    n_blk = len(blocks)
    wt = np.zeros((n_mm, 128, 128), dtype=np.float32)
    sel = np.zeros((n_blk, 128, 8), dtype=np.float32)
    fscale = np.zeros((8, 1), dtype=np.float32)
    sched = []                        # per block: (mrows, [(mm_idx, chunk)...])
    i = 0
    for b, blk in enumerate(blocks):
        mrows = blk["M"].shape[0]
        ent = []
        for c in blk["chunks"]:
            kw = CHW[c]
            wt[i, :kw, :mrows] = blk["M"][:, c * 128 : c * 128 + kw].T
            ent.append((i, c))
            i += 1
        for r, (f, sc) in enumerate(blk["feats"]):
            sel[b, r, f] = 1.0
            fscale[f, 0] = sc
        sched.append((mrows, ent))
    # device SBUF layout: partition-major [k, i, m] / [r, b, f]; one DMA each
    wt = np.ascontiguousarray(wt.transpose(1, 0, 2)).astype(NP_BF16)
    sel = np.ascontiguousarray(sel.transpose(1, 0, 2)).astype(NP_BF16)
    return wt, sel, fscale, sched


# ------------------------------------------------------------ device program
def _build_program(sched, n_mm, n_blk):
    nc = bacc.Bacc()
    xs_d = nc.dram_tensor("xs", [B_LOC, L_PAD], BF16, kind="ExternalInput")
    wt_d = nc.dram_tensor("wt", [128, n_mm, 128], BF16, kind="ExternalInput")
    sel_d = nc.dram_tensor("sel", [128, n_blk, 8], BF16, kind="ExternalInput")
    fs_d = nc.dram_tensor("fscale", [8, 1], F32, kind="ExternalInput")
    id_d = nc.dram_tensor("ident", [8, 8], F32, kind="ExternalInput")
    out_d = nc.dram_tensor("out", [B_LOC, 8], F32, kind="ExternalOutput")

    with tile.TileContext(nc) as tc:
        with (
            tc.tile_pool(name="const", bufs=1) as constp,
            tc.tile_pool(name="xt", bufs=3) as xtp,
            tc.tile_pool(name="za", bufs=6) as zap,
            tc.tile_pool(name="oute", bufs=2) as outp,
            tc.tile_pool(name="pz", bufs=4, space=bass.MemorySpace.PSUM) as pzp,
            tc.tile_pool(name="pf", bufs=2, space=bass.MemorySpace.PSUM) as pfp,
            tc.tile_pool(name="po", bufs=2, space=bass.MemorySpace.PSUM) as pop,
        ):
            wt_sb = constp.tile([128, n_mm, 128], BF16)
            nc.gpsimd.dma_start(wt_sb[:], wt_d[:])
            sel_sb = constp.tile([128, n_blk, 8], BF16)
            nc.gpsimd.dma_start(sel_sb[:], sel_d[:])
            fs_sb = constp.tile([8, 1], F32)
            nc.gpsimd.dma_start(fs_sb[:], fs_d[:])
            id_sb = constp.tile([8, 8], F32)
            nc.gpsimd.dma_start(id_sb[:], id_d[:])

            for t in range(TILES):
                trow = t * N_TILE
                # ---- xbar-transpose this tile's 512 samples from HBM
                xt_all = xtp.tile([128, NCH, N_TILE], BF16, tag="xt")
                for c in range(NCH):
                    nc.sync.dma_start(
                        xt_all[:, c, :],
                        xs_d[trow : trow + N_TILE, c * 128 : (c + 1) * 128],
                        transpose=True)

                # ---- banded matmuls, abs, per-block feature reduce
                pf = pfp.tile([8, N_TILE], F32, tag="pf")
                for b, (mrows, ent) in enumerate(sched):
                    pz = pzp.tile([mrows, N_TILE], F32, tag="pz")
                    for j, (i, c) in enumerate(ent):
                        kw = CHW[c]
                        nc.tensor.matmul(
                            pz[:],
                            wt_sb[0:kw, i, 0:mrows],
                            xt_all[0:kw, c, :],
                            start=(j == 0), stop=(j == len(ent) - 1),
                            skip_group_check=True)
                    za = zap.tile([mrows, N_TILE], BF16, tag="za")
                    nc.scalar.activation(
                        za[:], pz[:], mybir.ActivationFunctionType.Abs)
                    nc.tensor.matmul(
                        pf[:],
                        sel_sb[0:mrows, b, :],
                        za[:],
                        start=(b == 0), stop=(b == n_blk - 1),
                        skip_group_check=True)

                # ---- apply exact fp32 mean divisors, [8,512] -> [512,8], out
                fc = outp.tile([8, N_TILE], F32, tag="fc")
                nc.scalar.activation(
                    fc[:], pf[:], mybir.ActivationFunctionType.Copy,
                    scale=fs_sb[:])
                ob = outp.tile([128, GRPS, 8], F32, tag="ob")
                for g in range(GRPS):
                    po = pop.tile([128, 8], F32, tag="po")
                    nc.tensor.transpose(
                        po[:], fc[:, g * N_GRP : (g + 1) * N_GRP], id_sb[:])
                    nc.vector.tensor_copy(ob[:, g, :], po[:])
                nc.gpsimd.dma_start(
                    out_d[trow : trow + N_TILE, :].rearrange(
                        "(g p) f -> p g f", p=N_GRP),
                    ob[:])
    nc.finalize()
    return nc


_CACHE = {}


def _get_program(feat_w, pass_w):
    maps = _build_composite(feat_w, pass_w)
    blocks = _pack_blocks(maps)
    wt, sel, fscale, sched = _build_operands(blocks)
    key = tuple((m, tuple(e)) for m, e in sched)
    if key not in _CACHE:
        _CACHE[key] = _build_program(sched, wt.shape[1], sel.shape[1])
    return _CACHE[key], wt, sel, fscale


def kernel(x, feat_w, pass_w):
    nc, wt, sel, fscale = _get_program(feat_w, pass_w)
    ident = np.eye(8, dtype=np.float32)
    xs = np.zeros((B_FULL, L_PAD), dtype=NP_BF16)
    xs[:, :L_IN] = np.asarray(x, dtype=np.float32).reshape(B_FULL, L_IN)
    in_maps = [
        {"xs": xs[i * B_LOC : (i + 1) * B_LOC],
         "wt": wt, "sel": sel, "fscale": fscale, "ident": ident}
        for i in range(N_CORES)
    ]
    res = run_bass_kernel_spmd(nc, in_maps, list(range(N_CORES)))
    out = np.concatenate([res.results[i]["out"] for i in range(N_CORES)], axis=0)
    return np.ascontiguousarray(out.astype(np.float32))
